# revision 2
# baseline (speedup 1.0000x reference)
"""GCNConv (rank-1 normalized aggregation) Trainium2 kernel, SPMD over 8 cores.

Math (faithful to the torch/jax reference):
    h    = x @ W
    adj  = symmetric 0/1 adjacency from edge_index (duplicates collapse: SET, not add)
    deg  = adj.sum(1);  dinv = 1/sqrt(deg)
    agg  = dinv @ h                      # rank-1 identity, [F_OUT]
    out  = dinv[:, None] * agg[None, :] + bias

Since agg = (dinv @ x) @ W, the device never materializes h:
    per-core (rows sharded):  v_c = dinv_c @ x_c            ([F_IN], TensorE)
    AllReduce(v)  -> v                                       (512B collective)
    agg = v @ W                                              (TensorE)
    out_c = dinv_c (x) agg + bias   via one K=2 matmul per 128-row tile:
            lhsT = [dinv_c ; ones] (2 x rows), rhs = [agg ; bias] (2 x F_OUT)

The exact deduplicated degree (an integer/sorting problem, not a flops problem)
is computed on host with np.unique; all O(N*F) floating-point work runs on the
8 NeuronCores.
"""

import numpy as np

N, F_IN, F_OUT = 12000, 128, 256
N_CORES = 8
ROWS = N // N_CORES          # 1500 rows per core
NT = (ROWS + 127) // 128     # 12 row tiles per core
ROWS_PAD = NT * 128          # 1536 (zero-padded)

_cache = {}


def _build_nc():
    import concourse.bacc as bacc
    import concourse.mybir as mybir
    import concourse.tile as tile

    f32 = mybir.dt.float32

    nc = bacc.Bacc(
        "TRN2",
        target_bir_lowering=False,
        debug=False,
        num_devices=N_CORES,
    )

    x_d = nc.dram_tensor("x", [ROWS_PAD, F_IN], f32, kind="ExternalInput")
    dinv_d = nc.dram_tensor("dinv", [ROWS_PAD], f32, kind="ExternalInput")
    w_d = nc.dram_tensor("weight", [F_IN, F_OUT], f32, kind="ExternalInput")
    b_d = nc.dram_tensor("bias", [F_OUT], f32, kind="ExternalInput")
    out_d = nc.dram_tensor("out", [ROWS_PAD, F_OUT], f32, kind="ExternalOutput")

    x_tiles = x_d.ap().rearrange("(n p) m -> n p m", p=128)      # [NT,128,F_IN]
    out_tiles = out_d.ap().rearrange("(n p) m -> n p m", p=128)  # [NT,128,F_OUT]

    with tile.TileContext(nc) as tc:
        with (
            tc.tile_pool(name="const", bufs=1) as cpool,
            tc.tile_pool(name="xbuf", bufs=3) as xpool,
            tc.tile_pool(name="obuf", bufs=3) as opool,
            tc.tile_pool(name="vps", bufs=1, space="PSUM") as vpsum,
            tc.tile_pool(name="ops", bufs=4, space="PSUM") as opsum,
            tc.tile_pool(name="dram", bufs=1, space="DRAM") as dram,
        ):
            # ---- constants / small loads ----
            w_s = cpool.tile([F_IN, F_OUT], f32)
            nc.sync.dma_start(w_s[:], w_d.ap())

            # dinvT[p, f] = dinv[f*128 + p]  (column f = lhsT for row-tile f)
            dinvT = cpool.tile([128, NT], f32)
            nc.sync.dma_start(dinvT[:], dinv_d.ap().rearrange("(f p) -> p f", p=128))

            # stack: row0 = dinv (linear along free), row1 = ones
            stack = cpool.tile([2, ROWS_PAD], f32)
            nc.vector.memset(stack[:], 1.0)
            nc.sync.dma_start(
                stack[0:1, :], dinv_d.ap().rearrange("(a n) -> a n", a=1)
            )

            # rhs2: row0 = agg (filled later), row1 = bias
            rhs2 = cpool.tile([2, F_OUT], f32)
            nc.sync.dma_start(rhs2[1:2, :], b_d.ap().rearrange("(a n) -> a n", a=1))

            # ---- v_partial = dinv_shard @ x_shard  (accumulate over row tiles) ----
            pv = vpsum.tile([1, F_IN], f32)
            for i in range(NT):
                xt = xpool.tile([128, F_IN], f32)
                nc.sync.dma_start(xt[:], x_tiles[i])
                nc.tensor.matmul(
                    pv[:], dinvT[:, i : i + 1], xt[:],
                    start=(i == 0), stop=(i == NT - 1),
                )
            v_s = cpool.tile([1, F_IN], f32)
            nc.vector.tensor_copy(v_s[:], pv[:])

            # ---- AllReduce v across the 8 cores ----
            vin = dram.tile([1, F_IN], f32)
            vout = dram.tile([F_IN, 1], f32)
            nc.sync.dma_start(vin[:], v_s[:])
            nc.gpsimd.collective_compute(
                "AllReduce",
                mybir.AluOpType.add,
                replica_groups=[list(range(N_CORES))],
                ins=[vin.opt()],
                outs=[vout.opt()],
            )
            vcol = cpool.tile([F_IN, 1], f32)
            nc.sync.dma_start(vcol[:], vout[:])

            # ---- agg = v @ W ----
            pagg = vpsum.tile([1, F_OUT], f32)
            nc.tensor.matmul(pagg[:], vcol[:], w_s[:], start=True, stop=True)
            nc.vector.tensor_copy(rhs2[0:1, :], pagg[:])

            # ---- out tile i = dinv_i (x) agg + 1 (x) bias  (K=2 matmul) ----
            for i in range(NT):
                po = opsum.tile([128, F_OUT], f32)
                nc.tensor.matmul(
                    po[:], stack[:, i * 128 : (i + 1) * 128], rhs2[:],
                    start=True, stop=True,
                )
                ot = opool.tile([128, F_OUT], f32)
                nc.vector.tensor_copy(ot[:], po[:])
                nc.sync.dma_start(out_tiles[i], ot[:])

    nc.compile()
    return nc


def _get_nc():
    if "nc" not in _cache:
        _cache["nc"] = _build_nc()
    return _cache["nc"]


def _host_dinv(edge_index: np.ndarray) -> np.ndarray:
    """Exact deduplicated symmetric degree -> 1/sqrt(deg), matching
    adj[a,b]=1; adj[b,a]=1; deg=adj.sum(1)."""
    a = edge_index[0].astype(np.int64)
    b = edge_index[1].astype(np.int64)
    keys = np.unique(np.concatenate([a * N + b, b * N + a]))
    deg = np.bincount(keys // N, minlength=N).astype(np.float32)
    with np.errstate(divide="ignore"):
        dinv = (np.float32(1.0) / np.sqrt(deg)).astype(np.float32)
    return dinv


def kernel(x, edge_index, weight, bias, _trace=False):
    from concourse import bass_utils

    x = np.ascontiguousarray(x, dtype=np.float32)
    weight = np.ascontiguousarray(weight, dtype=np.float32)
    bias = np.ascontiguousarray(bias, dtype=np.float32)
    dinv = _host_dinv(np.asarray(edge_index))

    nc = _get_nc()

    in_maps = []
    for c in range(N_CORES):
        r0 = c * ROWS
        xp = np.zeros((ROWS_PAD, F_IN), np.float32)
        xp[:ROWS] = x[r0 : r0 + ROWS]
        dp = np.zeros((ROWS_PAD,), np.float32)
        dp[:ROWS] = dinv[r0 : r0 + ROWS]
        in_maps.append({"x": xp, "dinv": dp, "weight": weight, "bias": bias})

    res = bass_utils.run_bass_kernel_spmd(
        nc, in_maps, core_ids=list(range(N_CORES)), trace=_trace
    )
    out = np.concatenate(
        [res.results[c]["out"][:ROWS] for c in range(N_CORES)], axis=0
    )
    if _trace:
        _cache["last_results"] = res
    return out


# revision 52
# speedup vs baseline: 2.2875x; 2.2875x over previous
"""GCNConv (rank-1 normalized aggregation) Trainium2 kernel, SPMD over 8 cores.

Math (faithful to the torch/jax reference):
    h    = x @ W
    adj  = symmetric 0/1 adjacency from edge_index (duplicates collapse: SET, not add)
    deg  = adj.sum(1);  dinv = 1/sqrt(deg)
    agg  = dinv @ h                      # rank-1 identity, [F_OUT]
    out  = dinv[:, None] * agg[None, :] + bias

Since agg = (dinv @ x) @ W, h is never materialized:
    v    = dinv @ x            ([F_IN] weighted row-sum, DVE mul + strided reduce)
    agg  = v @ W               (TensorE)
    out_c = dinv_c (x) agg + bias     (rows sharded across cores)

Collectives in this environment have a ~55us fixed latency (measured with a
bare 512B AllReduce), far above the 8-core floor, so instead of sharding the
v-reduction + AllReduce, every core reads the full x (6.1MB, ~17us at HBM BW)
and computes v locally; only the O(N*F_OUT) output is sharded.

The exact deduplicated degree (an integer/sorting problem, not a flops
problem) is computed on host with np.unique; all O(N*F) floating-point work
runs on the NeuronCores.
"""

import numpy as np

N, F_IN, F_OUT = 12000, 128, 256
N_CORES = 8
ROWS = N // N_CORES            # 1500 output rows per core
NT_OUT = 12                    # 12 row tiles per core (padded)
ROWS_PAD = NT_OUT * 128        # 1536
NT_FULL = 96                   # full-x row tiles (padded)
N_PAD = NT_FULL * 128          # 12288
# x rows-per-partition per DMA/compute chunk; small first chunks so DVE
# starts sooner, ramping up once the pipeline is primed
CHUNK_SIZES = [4, 8, 16, 20, 24, 24]
N_CHUNKS = len(CHUNK_SIZES)

_cache = {}


def _build_nc():
    import concourse.bacc as bacc
    import concourse.mybir as mybir
    import concourse.tile as tile

    f32 = mybir.dt.float32
    bf16 = mybir.dt.bfloat16

    nc = bacc.Bacc(
        "TRN2",
        target_bir_lowering=False,
        debug=False,
        num_devices=N_CORES,
    )

    # x and dinvT travel as bf16: halves DMA bytes and DVE mul time; the
    # ~0.3% relative error on v is far inside the 2e-2 gate
    x_d = nc.dram_tensor("x", [N_PAD, F_IN], bf16, kind="ExternalInput")
    # dinvT[p, r] = dinv[p*96 + r] (host-prepared layout matching x view)
    dinvT_d = nc.dram_tensor("dinvT", [128, NT_FULL], bf16, kind="ExternalInput")
    dinvS_d = nc.dram_tensor("dinvS", [128, NT_OUT], f32, kind="ExternalInput")
    w_d = nc.dram_tensor("weight", [F_IN, F_OUT], bf16, kind="ExternalInput")
    b_d = nc.dram_tensor("bias", [F_OUT], f32, kind="ExternalInput")
    out_d = nc.dram_tensor("out", [ROWS_PAD, F_OUT], f32, kind="ExternalOutput")

    # x view: partition p holds rows [p*96, (p+1)*96) -> one contiguous 48KB
    # read per partition (vs 2048 scattered 512B runs for the (n p) m view)
    x_prm = x_d.ap().rearrange("(p r) m -> p r m", p=128)      # [128,96,128]
    out_pnm = out_d.ap().rearrange("(n p) m -> p n m", p=128)  # [128,12,256]

    dma_engines = [nc.sync, nc.scalar]

    with tile.TileContext(nc) as tc:
        with (
            tc.tile_pool(name="const", bufs=1) as cpool,
            tc.tile_pool(name="xbuf", bufs=1) as xpool,
            tc.tile_pool(name="scl", bufs=3) as spool,
            tc.tile_pool(name="obuf", bufs=1) as opool,
            tc.tile_pool(name="ps", bufs=1, space="PSUM") as psum,
        ):
            # ---- small constants first (cheap), then x chunks ----
            # (keep everything off gpsimd: SWDGE completion latency is ~9us
            # and its drain blocks dependents)
            dinvT = cpool.tile([128, NT_FULL], bf16)
            nc.sync.dma_start(dinvT[:], dinvT_d.ap())
            bias_s = cpool.tile([1, F_OUT], f32)
            nc.scalar.dma_start(bias_s[:], b_d.ap().rearrange("(a n) -> a n", a=1))

            xc = []
            off = 0
            offs = []
            for q in range(N_CHUNKS):
                sz = CHUNK_SIZES[q]
                t = xpool.tile([128, sz, F_IN], bf16, tag=f"xc{q}", name=f"xc{q}")
                dma_engines[q % len(dma_engines)].dma_start(
                    t[:], x_prm[:, off : off + sz, :]
                )
                xc.append(t)
                offs.append(off)
                off += sz

            # needed only mid/late kernel; queue after the x chunks
            dinvS = cpool.tile([128, NT_OUT], f32)
            nc.scalar.dma_start(dinvS[:], dinvS_d.ap())
            w_s = cpool.tile([F_IN, F_OUT], bf16)
            nc.sync.dma_start(w_s[:], w_d.ap())


            ones_col = cpool.tile([128, 1], bf16)
            nc.vector.memset(ones_col[:], 1.0)
            ones_row = cpool.tile([1, 128], f32)
            nc.vector.memset(ones_row[:], 1.0)

            # ---- v = dinv @ x ----
            # per chunk: scaled = x * dinv (DVE); TensorE contracts partitions
            # via ones-matmuls, ALL accumulating into one [1,512] PSUM bank:
            # pvw[0, u] = sum over rows r with r%4 == u//128 of dinv_r*x[r, u%128]
            pvw = psum.tile([1, 512], f32)
            total_sl = sum(CHUNK_SIZES) * F_IN // 512
            sl = 0
            for q in range(N_CHUNKS):
                sz = CHUNK_SIZES[q]
                d_bc = (
                    dinvT[:, offs[q] : offs[q] + sz]
                    .unsqueeze(2)
                    .broadcast_to([128, sz, F_IN])
                )
                scaled = spool.tile([128, sz, F_IN], bf16, tag=f"scaled{q % 3}",
                                    name=f"scaled{q}")
                nc.vector.tensor_mul(scaled[:], xc[q][:], d_bc)
                flat = scaled[:].rearrange("p t j -> p (t j)")
                for s in range((sz * F_IN) // 512):
                    nc.tensor.matmul(
                        pvw[:],
                        ones_col[:],
                        flat[:, s * 512 : (s + 1) * 512],
                        start=(sl == 0),
                        stop=(sl == total_sl - 1),
                        skip_group_check=True,
                    )
                    sl += 1
            # fold the 4 t-mod groups: one small strided reduce
            vrow = cpool.tile([1, F_IN], f32)
            nc.vector.tensor_reduce(
                vrow[:],
                pvw[:].rearrange("a (t j) -> a j t", j=F_IN),
                axis=mybir.AxisListType.X,
                op=mybir.AluOpType.add,
            )

            # v [1,128] -> vcol [128,1] via TensorE transpose; cast to bf16
            # (for the A2 matmul whose rhs W is bf16) in the PSUM->SBUF copy
            pvcol = psum.tile([F_IN, 1], f32)
            nc.tensor.transpose(pvcol[:], vrow[:], ones_row[:1, :1])
            vcol = cpool.tile([F_IN, 1], bf16)
            nc.vector.tensor_copy(vcol[:], pvcol[:])

            # ---- A2[p, o] = agg[o] = sum_j v[j] W[j, o]  (v bcast as lhsT) ----
            pA2 = psum.tile([128, F_OUT], f32)
            nc.tensor.matmul(
                pA2[:],
                vcol[:].broadcast_to([F_IN, 128]),
                w_s[:],
                start=True,
                stop=True,
            )
            A2 = cpool.tile([128, F_OUT], f32)
            nc.vector.tensor_copy(A2[:], pA2[:])
            pB2 = psum.tile([128, F_OUT], f32)
            nc.tensor.matmul(pB2[:], ones_row[:], bias_s[:], start=True, stop=True)
            B2 = cpool.tile([128, F_OUT], f32)
            nc.vector.tensor_copy(B2[:], pB2[:])

            # ---- out tile i = (A2 * dinvS_i) + B2, one fused DVE op each ----
            out_engines = [nc.sync, nc.scalar]
            n_og = 6
            per_dma = NT_OUT // n_og  # 2 tiles per output DMA
            for g in range(n_og):
                og = opool.tile([128, per_dma, F_OUT], f32, tag=f"og{g}",
                                name=f"og{g}")
                for j in range(per_dma):
                    i = g * per_dma + j
                    nc.vector.scalar_tensor_tensor(
                        og[:, j, :],
                        A2[:],
                        dinvS[:, i : i + 1],
                        B2[:],
                        op0=mybir.AluOpType.mult,
                        op1=mybir.AluOpType.add,
                    )
                out_engines[g % 2].dma_start(
                    out_pnm[:, g * per_dma : (g + 1) * per_dma, :], og[:]
                )

    nc.compile()
    return nc


def _get_nc():
    if "nc" not in _cache:
        _cache["nc"] = _build_nc()
    return _cache["nc"]


def _host_dinv(edge_index: np.ndarray) -> np.ndarray:
    """Exact deduplicated symmetric degree -> 1/sqrt(deg), matching
    adj[a,b]=1; adj[b,a]=1; deg=adj.sum(1)."""
    a = edge_index[0].astype(np.int64)
    b = edge_index[1].astype(np.int64)
    keys = np.unique(np.concatenate([a * N + b, b * N + a]))
    deg = np.bincount(keys // N, minlength=N).astype(np.float32)
    with np.errstate(divide="ignore"):
        dinv = (np.float32(1.0) / np.sqrt(deg)).astype(np.float32)
    return dinv


def kernel(x, edge_index, weight, bias, _trace=False):
    from concourse import bass_utils

    x = np.ascontiguousarray(x, dtype=np.float32)
    weight = np.ascontiguousarray(weight, dtype=np.float32)
    bias = np.ascontiguousarray(bias, dtype=np.float32)
    dinv = _host_dinv(np.asarray(edge_index))

    nc = _get_nc()

    import ml_dtypes

    bf16 = ml_dtypes.bfloat16
    xp = np.zeros((N_PAD, F_IN), bf16)
    xp[:N] = x.astype(bf16)
    dp = np.zeros((N_PAD,), np.float32)
    dp[:N] = dinv
    # dinvT[p, r] = dinv[p*96 + r], matching the x view "(p r) m -> p r m"
    dinvT = np.ascontiguousarray(dp.reshape(128, NT_FULL)).astype(bf16)

    w16 = weight.astype(bf16)
    in_maps = []
    for c in range(N_CORES):
        r0 = c * ROWS
        ds = np.zeros((ROWS_PAD,), np.float32)
        ds[:ROWS] = dinv[r0 : r0 + ROWS]
        dinvS = np.ascontiguousarray(ds.reshape(NT_OUT, 128).T)  # [128, 12]
        in_maps.append(
            {
                "x": xp,
                "dinvT": dinvT,
                "dinvS": dinvS,
                "weight": w16,
                "bias": bias,
            }
        )

    res = bass_utils.run_bass_kernel_spmd(
        nc, in_maps, core_ids=list(range(N_CORES)), trace=_trace
    )
    out = np.concatenate(
        [res.results[c]["out"][:ROWS] for c in range(N_CORES)], axis=0
    )
    if _trace:
        _cache["last_results"] = res
    return out
